# revision 24
# baseline (speedup 1.0000x reference)
"""Trainium2 Bass kernel for nn_Attention_8839042695176.

Full (unsharded) inputs in, full output out. Internally: 8 NeuronCores,
core h owns attention head h (both batch elements), convs/qkv replicated
per core on that core's permuted channel order.

Math per (b, h) unit:
    scores[i,j] = q_full[c,i]·emb[c,j] + qd_up[c,i]·kd_up[c,j]   (K=16 matmul)
    attn = softmax_j(scores)        (no max-subtraction; |scores| <~ 8)
    out[c,i]  = sum_j attn[i,j] vv[c,j]
computed in transposed layout E^T[j,i] so both big matmuls stream on PE,
with the softmax denominator fused in as an extra all-ones row of vv^T.

v2 scheduling: phase A builds batch-0's operands fully (qkv both batches,
convs both batches); the batch-1 tail (gelu, upsample rows, emb DMA) is
interleaved into batch-0's ACT-bound main loop. Gelu is the tanh
approximation (Square/Tanh share the Exp activation table set, so the
main loop never reloads tables). S/R rows are laid out [up 0-7; q/emb
8-15] with head-q channels permuted to rows 8-15 so every S row is
written by an engine copy or a direct DRAM DMA (no SBUF gather DMAs).
"""

import numpy as np

HEADS = 8
DIM_HEAD = 8
B = 2
C = 64
H = 48
HW = H * H          # 2304
KS = 11             # conv kernel
STRIDE = 8
PAD = 2
M6 = 6              # downsampled side
MM = M6 * M6        # 36
PADW = H + 2 * PAD  # 52
PADHW = PADW * PADW # 2704
SCALE = DIM_HEAD ** (-0.5)
NCORES = 8
TAPS = KS * KS      # 121
GELU_C = 0.044715
GELU_S = 0.7978845608028654  # sqrt(2/pi)

# i-chunks for the main loop (<=512 fp32 moving limit)
ICHUNKS = [(0, 512), (512, 512), (1024, 512), (1536, 512), (2048, 256)]
# chunks for the qkv projection, row-aligned to 48 (10 rows / 8 rows)
QCHUNKS = [(0, 480), (480, 480), (960, 480), (1440, 480), (1920, 384)]
NJT = HW // 128     # 18 j-tiles

_PROGRAMS = {}
_DEBUG = False


def _build_program(repeat=1, split=True):
    from contextlib import ExitStack
    import concourse.bass as bass
    import concourse.mybir as mybir
    import concourse.tile as tile
    from concourse.masks import make_identity

    F32 = mybir.dt.float32
    BF = mybir.dt.bfloat16
    AF = mybir.ActivationFunctionType

    nc = bass.Bass(trn_type="TRN2")

    f2 = nc.declare_dram_parameter("f2", [B, C, HW], BF, isOutput=False)
    wAll = nc.declare_dram_parameter("wAll", [C, 136], BF, isOutput=False)
    wqT = nc.declare_dram_parameter("wqT", [128, 66 * 8], BF, isOutput=False)
    wkT = nc.declare_dram_parameter("wkT", [128, 66 * 8], BF, isOutput=False)
    bqk = nc.declare_dram_parameter("bqk", [8, 2], F32, isOutput=False)
    emb = nc.declare_dram_parameter("emb", [8, HW], BF, isOutput=False)
    out = nc.declare_dram_parameter("out", [B, 8, HW], F32, isOutput=True)
    dbg = {}
    if _DEBUG:
        for name, shape in [("d_xq", [128, B, PADHW]), ("d_xk", [128, B, PADHW]),
                            ("d_vv", [8, B, HW]), ("d_qd", [8, B, MM]),
                            ("d_kd", [8, B, MM]), ("d_s0", [16, HW]),
                            ("d_r0", [16, HW]), ("d_vt0", [128, NJT * 9])]:
            dbg[name] = nc.declare_dram_parameter(name, shape, F32, isOutput=True)

    def interior(Xt, b, r0=0):
        """[*, 48, 48] strided view of the padded map's valid region."""
        return bass.AP(
            tensor=Xt.tensor,
            offset=Xt.offset + r0 * Xt.ap[0][0] + b * PADHW + PAD * PADW + PAD,
            ap=[[Xt.ap[0][0], 8], [PADW, H], [1, H]],
        )

    def up_ap(Dt, b):
        """Broadcast view: D[c, b, p] -> [c, 36(p), 64(repeat)]."""
        return bass.AP(
            tensor=Dt.tensor,
            offset=Dt.offset + b * MM,
            ap=[[Dt.ap[0][0], 8], [1, MM], [0, 64]],
        )

    # conv tap slots: kx=10 singles first (no shift-row dependency), then
    # the 5 (kx,kx+1) pairs per ky row
    SLOTS = []
    for ky in range(KS):
        SLOTS.append((ky, 10, False))
    for ky in range(KS):
        for pk in range(5):
            SLOTS.append((ky, 2 * pk, True))

    with tile.TileContext(nc) as tc, ExitStack() as ctx:
        const = ctx.enter_context(tc.tile_pool(name="const", bufs=1))
        work = ctx.enter_context(tc.tile_pool(name="work", bufs=3))
        epool = ctx.enter_context(tc.tile_pool(name="epool", bufs=3))

        ID8 = const.tile([8, 8], BF)
        make_identity(nc, ID8)
        ONE9 = const.tile([1, 9], F32)
        nc.vector.memset(ONE9, 1.0)
        WARM = const.tile([1, 9], F32)
        nc.scalar.activation(WARM, ONE9, mybir.ActivationFunctionType.Exp)
        WARMT = const.tile([8, 480], BF)
        nc.vector.memset(WARMT, 1.0)

        def _rep_body():
            # ---- persistent constants (SP queue order matters) ----
            WA = const.tile([C, 136], BF)
            nc.sync.dma_start(WA, wAll[:, :])
            W1 = WA[:, 0:128]
            WV = WA[:, 128:136]
            BQK = const.tile([8, 2], F32)
            QD = const.tile([8, B, MM], F32)
            KD = const.tile([8, B, MM], F32)
            Ss = [const.tile([16, HW], BF, name=f"S{b}") for b in range(B)]
            Rs = [const.tile([16, HW], BF, name=f"R{b}") for b in range(B)]
            VTs = [const.tile([128, NJT, 9], BF, name=f"VT{b}") for b in range(B)]
            WcQ = const.tile([128, 66 * 8], BF)
            WcK = const.tile([128, 66 * 8], BF)
            for b in range(B):
                nc.vector.memset(VTs[b][:, :, 0:1], 1.0)

            with tc.tile_pool(name="stage", bufs=1) as stage:
                XQ = stage.tile([128, B, PADHW], BF)
                XK = stage.tile([128, B, PADHW], BF)
                VV = stage.tile([8, B, HW], BF)
                F = stage.tile([C, B, HW], BF)
                UPS = stage.tile([8, HW], BF, name="UPS")
                UPR = stage.tile([8, HW], BF, name="UPR")
                nc.sync.dma_start(F[:, 0, :], f2[0, :, :])
                nc.scalar.dma_start(F[:, 1, :], f2[1, :, :])
                nc.sync.dma_start(WcQ, wqT[:, :])
                nc.gpsimd.dma_start(WcK, wkT[:, :])
                nc.sync.dma_start(BQK, bqk[:, :])
                BQ = BQK[:, 0:1]
                BK = BQK[:, 1:2]

                # zero the padding border of rows 0-63 (rows 64-127 come from
                # the shift DMA, borders included)
                for Xt in (XQ, XK):
                    for b in range(B):
                        o = Xt.offset + b * PADHW
                        pap = [[Xt.ap[0][0], 64]]
                        nc.gpsimd.memset(
                            bass.AP(tensor=Xt.tensor, offset=o,
                                    ap=pap + [[1, 2 * PADW + PAD]]), 0.0)
                        nc.gpsimd.memset(
                            bass.AP(tensor=Xt.tensor,
                                    offset=o + (H + PAD - 1) * PADW + PAD + H,
                                    ap=pap + [[1, 2 * PADW + PAD]]), 0.0)
                        nc.gpsimd.memset(
                            bass.AP(tensor=Xt.tensor,
                                    offset=o + PAD * PADW + PAD + H,
                                    ap=pap + [[PADW, H - 1], [1, 2 * PAD]]), 0.0)

                def shift_dma(eng, Xt, b):
                    """rows 64-127 = rows 0-63 shifted left one element."""
                    src = bass.AP(tensor=Xt.tensor,
                                  offset=Xt.offset + b * PADHW + 1,
                                  ap=[[Xt.ap[0][0], 64], [1, PADHW - 1]])
                    dst = bass.AP(tensor=Xt.tensor,
                                  offset=Xt.offset + 64 * Xt.ap[0][0] + b * PADHW,
                                  ap=[[Xt.ap[0][0], 64], [1, PADHW - 1]])
                    eng.dma_start(dst, src)

                def gelu_chain(acc, b, Bt, Dt, cid=[0]):
                    """tanh-gelu of (acc[:, b, :] + Bt) -> Dt[:, b, :] (x2 scale:
                    the 0.5 is folded into the UPQ scale downstream)."""
                    cid[0] += 1
                    sfx = str(cid[0])
                    xg = work.tile([8, MM], F32, tag="gx" + sfx, name="gx")
                    nc.scalar.activation(xg, acc[:, b, :], AF.Identity, bias=Bt)
                    sq = work.tile([8, MM], F32, tag="gs" + sfx, name="gs")
                    nc.scalar.activation(sq, acc[:, b, :], AF.Square, bias=Bt)
                    va = work.tile([8, MM], F32, tag="gv" + sfx, name="gv")
                    nc.vector.scalar_tensor_tensor(
                        va, in0=sq, scalar=1.0 / GELU_C, in1=xg,
                        op0=mybir.AluOpType.add, op1=mybir.AluOpType.mult)
                    th = work.tile([8, MM], F32, tag="gt" + sfx, name="gt")
                    nc.scalar.activation(th, va, AF.Tanh,
                                         scale=GELU_S * GELU_C)
                    nc.vector.scalar_tensor_tensor(
                        Dt[:, b, :], in0=th, scalar=1.0, in1=xg,
                        op0=mybir.AluOpType.add, op1=mybir.AluOpType.mult)

                def up_build(Dt, b, scratch, scale, dst, dma_eng):
                    """dst[8:16, :] = broadcast-64(Dt[:, b, :]) * scale, built
                    in scratch rows 0-7 by Pool/DVE halves, DMA'd per half."""
                    half = MM // 2
                    for eng, lo, hi in ((nc.gpsimd, 0, half),
                                        (nc.vector, half, MM)):
                        dv = bass.AP(tensor=scratch.tensor,
                                     offset=scratch.offset + lo * 64,
                                     ap=[[scratch.ap[0][0], 8],
                                         [64, hi - lo], [1, 64]])
                        sv = bass.AP(tensor=Dt.tensor,
                                     offset=Dt.offset + b * MM + lo,
                                     ap=[[Dt.ap[0][0], 8], [1, hi - lo], [0, 64]])
                        eng.tensor_scalar_mul(dv, sv, scale)
                        dma_eng.dma_start(dst[8:16, lo * 64:hi * 64],
                                          scratch[:, lo * 64:hi * 64])

                def sq_copy(eng, b):
                    """S rows 0-7 (head-q, flat) from the XQ interior view."""
                    src = interior(XQ, b)
                    dst = Ss[b][0:8, :].rearrange("p (h w) -> p h w", h=H, w=H)
                    if eng is nc.scalar:
                        eng.activation(dst, src, AF.Copy)
                    else:
                        eng.tensor_copy(dst, src)

                def build_vt(b, ppA):
                    for g in range(2):
                        pt9 = ppA.tile([128, 9, 8], BF, tag="pt", bufs=1,
                                       name="pt9")
                        for t in range(9):
                            jt = g * 9 + t
                            nc.tensor.transpose(
                                pt9[:, t, :], VV[:, b, jt * 128:(jt + 1) * 128],
                                ID8)
                        nc.vector.tensor_copy(
                            VTs[b][:, g * 9:(g + 1) * 9, 1:9], pt9)

                with tc.tile_pool(name="ppA", bufs=1, space="PSUM") as ppA, \
                     tc.tile_pool(name="ppB", bufs=1, space="PSUM") as ppB:
                    def pe_warm(n=1):
                        # dependency-free matmuls that run in PE idle gaps,
                        # holding the tensor engine's DVFS ramp
                        for _ in range(n):
                            wp = ppA.tile([8, 480], F32, tag="warm", bufs=1,
                                          name="wp")
                            nc.tensor.matmul(wp, lhsT=WARMT[:, 0:8],
                                             rhs=WARMT, start=True, stop=True)

                    # ---- qkv projection, both batches ----
                    for b in range(B):
                        for ci, (j0, nj) in enumerate(QCHUNKS):
                            nrows = nj // H
                            y0 = j0 // H
                            pq = ppA.tile([128, 480], F32, tag="pq", bufs=3)
                            nc.tensor.matmul(pq[:, :nj], lhsT=W1,
                                             rhs=F[:, b, j0:j0 + nj],
                                             start=True, stop=True)
                            pv = ppA.tile([8, 480], F32, tag="pv", bufs=1)
                            nc.tensor.matmul(pv[:, :nj], lhsT=WV,
                                             rhs=F[:, b, j0:j0 + nj],
                                             start=True, stop=True)
                            for Xt, r0, eng in ((XQ, 0, nc.scalar),
                                                (XK, 64, nc.vector)):
                                src = pq[r0:r0 + 64, :nj].rearrange(
                                    "p (r w) -> p r w", r=nrows, w=H)
                                dst = bass.AP(
                                    tensor=Xt.tensor,
                                    offset=(Xt.offset + b * PADHW
                                            + (PAD + y0) * PADW + PAD),
                                    ap=[[Xt.ap[0][0], 64], [PADW, nrows],
                                        [1, H]])
                                if eng is nc.scalar:
                                    eng.activation(dst, src, AF.Copy)
                                else:
                                    eng.tensor_copy(dst, src)
                            # gpsimd cannot read PSUM on HW; split across
                            # ACT (b0) and DVE (b1)
                            if b == 0:
                                nc.scalar.activation(VV[:, b, j0:j0 + nj],
                                                     pv[:, :nj], AF.Copy)
                            else:
                                nc.vector.tensor_copy(VV[:, b, j0:j0 + nj],
                                                      pv[:, :nj])
                            pe_warm(2)
                        if b == 0:
                            shift_dma(nc.sync, XQ, 0)
                            shift_dma(nc.gpsimd, XK, 0)
                        else:
                            shift_dma(nc.sync, XQ, 1)
                            shift_dma(nc.gpsimd, XK, 1)
                    nc.sync.dma_start(Rs[0][0:8, :], emb[:, :])

                    # ---- strided 11x11 convs, q fully then k, so the
                    # q-side gelu/up/DMA tail overlaps the k-side taps ----
                    accQ = ppB.tile([8, B, MM], F32, tag="accq")
                    accK = ppB.tile([8, B, MM], F32, tag="acck")

                    def conv(Xt, Wc, acc):
                        for si, (ky, kx, paired) in enumerate(SLOTS):
                            kp = 128 if paired else 64
                            rhs = bass.AP(
                                tensor=Xt.tensor,
                                offset=Xt.offset + ky * PADW + kx,
                                ap=[[Xt.ap[0][0], kp], [PADHW, B],
                                    [STRIDE * PADW, M6], [STRIDE, M6]])
                            nc.tensor.matmul(
                                acc, lhsT=Wc[0:kp, si * 8:(si + 1) * 8],
                                rhs=rhs,
                                start=(si == 0), stop=(si == len(SLOTS) - 1))

                    conv(XQ, WcQ, accQ)
                    gelu_chain(accQ, 0, BQ, QD)
                    gelu_chain(accQ, 1, BQ, QD)
                    up_build(QD, 0, UPS, SCALE * 0.25, Ss[0], nc.sync)

                    conv(XK, WcK, accK)
                    gelu_chain(accK, 0, BK, KD)
                    gelu_chain(accK, 1, BK, KD)
                    up_build(KD, 0, UPR, 1.0, Rs[0], nc.gpsimd)

                    # S flat q-rows (off conv critical path)
                    sq_copy(nc.scalar, 0)
                    sq_copy(nc.vector, 1)

                    build_vt(0, ppA)
                    build_vt(1, ppA)
                    for _ in range(24):
                        wp = ppA.tile([8, 480], F32, tag="warm", bufs=1,
                                      name="wp")
                        nc.tensor.matmul(wp, lhsT=UPS[0:8, 0:8],
                                         rhs=UPS[:, 0:480], start=True,
                                         stop=True)

                    if _DEBUG:
                        nc.sync.dma_start(dbg["d_xq"][:, :, :], XQ)
                        nc.sync.dma_start(dbg["d_xk"][:, :, :], XK)
                        nc.sync.dma_start(dbg["d_vv"][:, :, :], VV)

                # ---- main loops; batch-1 up rows interleaved into Main0 ----
                def bundle_b1():
                    yield lambda: nc.sync.dma_start(Rs[1][0:8, :], emb[:, :])
                    yield lambda: up_build(QD, 1, UPS, SCALE * 0.25, Ss[1],
                                           nc.sync)
                    yield lambda: up_build(KD, 1, UPR, 1.0, Rs[1], nc.gpsimd)

                bundles = bundle_b1()

                if _DEBUG:
                    for _ in bundles:
                        pass
                    bundles = iter(())
                    nc.sync.dma_start(dbg["d_qd"][:, :, :], QD)
                    nc.sync.dma_start(dbg["d_kd"][:, :, :], KD)
                    nc.sync.dma_start(dbg["d_s0"][:, :], Ss[0])
                    nc.sync.dma_start(dbg["d_r0"][:, :], Rs[0])
                    nc.sync.dma_start(dbg["d_vt0"][:, :],
                                      VTs[0].rearrange("p a b -> p (a b)"))

                NG = 3           # j-tiles per PSUM group
                with tc.tile_pool(name="psum_main", bufs=1, space="PSUM") as pm:
                    steps = [(b, i0, ni, jg)
                             for b in range(B)
                             for (i0, ni) in ICHUNKS
                             for jg in range(NJT // NG)]
                    po_cur = [None]
                    pending = [None]

                    def emit_o():
                        pb_, pi0, pni, pjg, pesb = pending[0]
                        if pjg == 0:
                            po_cur[0] = pm.tile([9, 512], F32, tag="po",
                                                bufs=2, name="po")
                        po = po_cur[0]
                        VT = VTs[pb_]
                        for t in range(NG):
                            nc.tensor.matmul(
                                po[:, :pni], lhsT=VT[:, NG * pjg + t, :],
                                rhs=pesb[:, t, :pni],
                                start=(pjg == 0 and t == 0),
                                stop=(pjg == NJT // NG - 1 and t == NG - 1))
                        if pjg == NJT // NG - 1:
                            rec = work.tile([1, 512], F32, tag="rec", name="rec")
                            nc.vector.reciprocal(rec[:, :pni], po[0:1, :pni])
                            pbs = work.tile([9, 512], F32, tag="pbs", name="pbs")
                            if (pb_, pi0) == (B - 1, ICHUNKS[-1][0]):
                                # tail: PE is idle, broadcast via ones-matmul
                                pbp = pm.tile([9, 512], F32, tag="po", bufs=2,
                                              name="pbp")
                                nc.tensor.matmul(pbp[:, :pni], lhsT=ONE9,
                                                 rhs=rec[:, :pni],
                                                 start=True, stop=True)
                                nc.vector.tensor_copy(pbs[:, :pni],
                                                      pbp[:, :pni])
                            else:
                                nc.sync.dma_start(
                                    pbs[:, :pni],
                                    bass.AP(tensor=rec.tensor,
                                            offset=rec.offset,
                                            ap=[[1, 1], [0, 9], [1, pni]]))
                            res = work.tile([9, 512], F32, tag="res", name="res")
                            nc.vector.tensor_mul(res[:, :pni], po[:, :pni],
                                                 pbs[:, :pni])
                            nc.sync.dma_start(out[pb_, :, pi0:pi0 + pni],
                                              res[1:9, :pni])

                    for sidx, step in enumerate(steps):
                        b, i0, ni, jg = step
                        S, R = Ss[b], Rs[b]
                        pe3 = pm.tile([128, NG, 512], F32, tag="pe", bufs=2,
                                      name="pe3")
                        for t in range(NG):
                            jt = NG * jg + t
                            nc.tensor.matmul(
                                pe3[:, t, :ni],
                                lhsT=R[:, jt * 128:(jt + 1) * 128],
                                rhs=S[:, i0:i0 + ni], start=True, stop=True)
                        esb3 = epool.tile([128, NG, 512], BF, tag="esb", bufs=4,
                                          name="esb3")
                        nc.scalar.activation(esb3[:, :, :ni], pe3[:, :, :ni],
                                             AF.Exp)
                        if pending[0] is not None:
                            emit_o()
                        pending[0] = (b, i0, ni, jg, esb3)
                        if sidx % 2 == 1:
                            nxt = next(bundles, None)
                            if nxt is not None:
                                nxt()
                    emit_o()

        for _rep in range(repeat):
            _rep_body()

    if split:
        _split_waits(nc)
    return nc


def _split_waits(nc):
    """This walrus build allows at most ONE sync-wait per instruction.
    Move excess waits onto same-engine NoOps inserted just before."""
    import concourse.mybir as mybir
    ctr = 0
    for fn in nc.m.functions:
        for blk in fn.blocks:
            new = []
            for inst in blk.instructions:
                si = inst.sync_info
                waits = list(si.on_wait) if si and si.on_wait else []
                if len(waits) > 1:
                    for w in waits[:-1]:
                        ctr += 1
                        nop = mybir.InstNoOp(name=f"I-wsplit-{ctr}", ins=[], outs=[])
                        nop.engine = inst.engine
                        nop.sync_info = mybir.SyncInfo(on_wait=[w], on_update=[])
                        new.append(nop)
                    inst.sync_info = mybir.SyncInfo(
                        on_wait=[waits[-1]],
                        on_update=list(si.on_update or []))
                new.append(inst)
            blk.instructions = new


def _get_program(repeat=1):
    if repeat not in _PROGRAMS:
        _PROGRAMS[repeat] = _build_program(repeat)
    return _PROGRAMS[repeat]


def _make_in_maps(f, w_qkv, wq, bq, wk, bk, pos_h, pos_w):
    import ml_dtypes
    BF = ml_dtypes.bfloat16
    f2 = np.ascontiguousarray(f.reshape(B, C, HW)).astype(BF)
    embv = np.ascontiguousarray(
        (pos_h[:, :, None] + pos_w[:, None, :]).reshape(8, HW)).astype(BF)
    w = w_qkv[:, :, 0, 0].astype(np.float32)
    wq = wq.astype(np.float32)
    wk = wk.astype(np.float32)
    in_maps = []
    for h in range(NCORES):
        head = np.arange(h * 8, h * 8 + 8)
        rest = np.delete(np.arange(C), head)
        perm = np.concatenate([head, rest])
        wAll = np.ascontiguousarray(np.concatenate(
            [w[0:C][perm].T, w[C:2 * C].T,
             w[2 * C + h * 8: 2 * C + h * 8 + 8].T], axis=1)).astype(BF)

        def pack_taps(wp):
            # [8 head-oc, ic', ky, kx] -> [128, 66*8], kx=10 singles first,
            # then (kx,kx+1) pairs; partner tap weights at rows 64-127.
            # Must match the kernel's SLOTS order.
            w2 = np.zeros((128, 66 * 8), np.float32)
            si = 0
            for ky in range(KS):
                w2[0:64, si * 8:(si + 1) * 8] = wp[:, :, ky, 10].T
                si += 1
            for ky in range(KS):
                for pk in range(5):
                    w2[0:64, si * 8:(si + 1) * 8] = wp[:, :, ky, 2 * pk].T
                    w2[64:128, si * 8:(si + 1) * 8] = wp[:, :, ky, 2 * pk + 1].T
                    si += 1
            return np.ascontiguousarray(w2).astype(BF)

        wqT = pack_taps(wq[head][:, perm])     # head out-rows, perm'd inputs
        wkT = pack_taps(wk[head])              # k-map channels in natural order
        in_maps.append({
            "f2": f2,
            "wAll": wAll,
            "wqT": wqT,
            "wkT": wkT,
            "bqk": np.ascontiguousarray(
                np.stack([bq[head], bk[head]], axis=1).astype(np.float32)),
            "emb": embv,
        })
    return in_maps


def _assemble(results):
    fmap = np.empty((B, C, HW), np.float32)
    for h in range(NCORES):
        fmap[:, h * 8:(h + 1) * 8, :] = results[h]["out"]
    return fmap.reshape(B, C, H, H)


def run(trace=False, **inputs):
    """Run on hardware; returns (output, BassKernelResults)."""
    from concourse.bass_utils import run_bass_kernel_spmd
    nc = _get_program()
    in_maps = _make_in_maps(**inputs)
    res = run_bass_kernel_spmd(nc, in_maps, core_ids=list(range(NCORES)),
                               trace=trace)
    return _assemble(res.results), res


def kernel(**inputs):
    out, _ = run(trace=False, **inputs)
    return out


# revision 29
# speedup vs baseline: 1.2640x; 1.2640x over previous
"""v3 + batched VT copies, engine S-rows, split UP builds."""

import numpy as np

HEADS = 8
DIM_HEAD = 8
B = 2
C = 64
H = 48
HW = H * H
KS = 11
STRIDE = 8
PAD = 2
M6 = 6
MM = M6 * M6
PADW = H + 2 * PAD
PADHW = PADW * PADW
SCALE = DIM_HEAD ** (-0.5)
NCORES = 8
TAPS = KS * KS

ICHUNKS = [(0, 512), (512, 512), (1024, 512), (1536, 512), (2048, 256)]
QCHUNKS = [(0, 480), (480, 480), (960, 480), (1440, 480), (1920, 384)]
NJT = HW // 128

_PROGRAMS = {}
_CONV_ACT = "Gelu"
_DEBUG = False


def _build_program(repeat=1, split=True):
    from contextlib import ExitStack
    import concourse.bass as bass
    import concourse.mybir as mybir
    import concourse.tile as tile
    from concourse.masks import make_identity

    F32 = mybir.dt.float32
    BF = mybir.dt.bfloat16
    AF = mybir.ActivationFunctionType

    nc = bass.Bass(trn_type="TRN2")

    f2 = nc.declare_dram_parameter("f2", [B, C, HW], BF, isOutput=False)
    wAll = nc.declare_dram_parameter("wAll", [C, 136], BF, isOutput=False)
    wqT = nc.declare_dram_parameter("wqT", [128, 66 * 8], BF, isOutput=False)
    wkT = nc.declare_dram_parameter("wkT", [128, 66 * 8], BF, isOutput=False)
    bqk = nc.declare_dram_parameter("bqk", [8, 2], F32, isOutput=False)
    emb = nc.declare_dram_parameter("emb", [8, HW], BF, isOutput=False)
    out = nc.declare_dram_parameter("out", [B, 8, HW], F32, isOutput=True)

    def interior(Xt, b):
        return bass.AP(
            tensor=Xt.tensor,
            offset=Xt.offset + b * PADHW + PAD * PADW + PAD,
            ap=[Xt.ap[0], [PADW, H], [1, H]],
        )

    def up_ap(Dt, b, nrows=8):
        return bass.AP(
            tensor=Dt.tensor,
            offset=Dt.offset + b * MM,
            ap=[[Dt.ap[0][0], nrows], [1, MM], [0, 64]],
        )

    with tile.TileContext(nc) as tc, ExitStack() as ctx:
        const = ctx.enter_context(tc.tile_pool(name="const", bufs=1))
        work = ctx.enter_context(tc.tile_pool(name="work", bufs=3))
        epool = ctx.enter_context(tc.tile_pool(name="epool", bufs=3))

        ID8 = const.tile([8, 8], BF)
        make_identity(nc, ID8)
        ONE9 = const.tile([1, 9], F32)
        nc.vector.memset(ONE9, 1.0)

        def _rep_body():
            WA = const.tile([C, 136], BF)
            nc.sync.dma_start(WA, wAll[:, :])
            W1 = WA[:, 0:128]
            WV = WA[:, 128:136]
            BQK = const.tile([8, 2], F32)
            nc.sync.dma_start(BQK, bqk[:, :])
            BQ = BQK[:, 0:1]
            BK = BQK[:, 1:2]
            WcQ = const.tile([128, 66 * 8], BF)
            nc.sync.dma_start(WcQ, wqT[:, :])
            WcK = const.tile([128, 66 * 8], BF)
            nc.sync.dma_start(WcK, wkT[:, :])
            QD = const.tile([8, B, MM], F32)
            KD = const.tile([8, B, MM], F32)
            Ss = [const.tile([16, HW], BF, name=f"S{b}") for b in range(B)]
            Rs = [const.tile([16, HW], BF, name=f"R{b}") for b in range(B)]
            VTs = [const.tile([128, NJT, 9], BF, name=f"VT{b}") for b in range(B)]

            with tc.tile_pool(name="stage", bufs=1) as stage, \
                 tc.tile_pool(name="psum_pro", bufs=1, space="PSUM") as pp:
                F = stage.tile([C, B, HW], BF)
                for b in range(B):
                    nc.sync.dma_start(F[:, b, :], f2[b, :, :])
                XQ = stage.tile([128, B, PADHW], BF)
                XK = stage.tile([128, B, PADHW], BF)
                VV = stage.tile([8, B, HW], BF)

                for Xt in (XQ, XK):
                    for b in range(B):
                        base = b * PADHW
                        for shift, poff in ((0, 0), (1, 64)):
                            o = Xt.offset + poff * Xt.ap[0][0] + base
                            pap = [[Xt.ap[0][0], 64]]
                            nc.gpsimd.memset(
                                bass.AP(tensor=Xt.tensor, offset=o,
                                        ap=pap + [[1, 2 * PADW + PAD - shift]]), 0.0)
                            nc.gpsimd.memset(
                                bass.AP(tensor=Xt.tensor,
                                        offset=o + (H + PAD - 1) * PADW + PAD + H - shift,
                                        ap=pap + [[1, 2 * PADW + PAD + shift]]), 0.0)
                            nc.gpsimd.memset(
                                bass.AP(tensor=Xt.tensor,
                                        offset=o + PAD * PADW + PAD + H - shift,
                                        ap=pap + [[PADW, H - 1], [1, 2 * PAD]]), 0.0)

                for b in range(B):
                    for (j0, nj) in QCHUNKS:
                        nrows = nj // H
                        y0 = j0 // H
                        pq = pp.tile([128, 480], F32, tag="pq", bufs=2)
                        nc.tensor.matmul(pq[:, :nj], lhsT=W1, rhs=F[:, b, j0:j0 + nj],
                                         start=True, stop=True)
                        pv = pp.tile([8, 480], F32, tag="pv", bufs=2)
                        nc.tensor.matmul(pv[:, :nj], lhsT=WV, rhs=F[:, b, j0:j0 + nj],
                                         start=True, stop=True)
                        for Xt, r0, eng in ((XQ, 0, nc.scalar),
                                            (XK, 64, nc.vector)):
                            src = pq[r0:r0 + 64, :nj].rearrange(
                                "p (r w) -> p r w", r=nrows, w=H)
                            for shift, poff in ((0, 0), (1, 64)):
                                dst = bass.AP(
                                    tensor=Xt.tensor,
                                    offset=(Xt.offset + poff * Xt.ap[0][0]
                                            + b * PADHW
                                            + (PAD + y0) * PADW + PAD - shift),
                                    ap=[[Xt.ap[0][0], 64], [PADW, nrows], [1, H]])
                                if eng is nc.scalar:
                                    eng.activation(dst, src, AF.Copy)
                                else:
                                    eng.tensor_copy(dst, src)
                        nc.vector.tensor_copy(VV[:, b, j0:j0 + nj], pv[:, :nj])

                GELU_C = 0.044715
                GELU_S = 0.7978845608028654

                def gelu_chain(acc, b, Bt, Dt, sfx):
                    xg = work.tile([8, MM], F32, tag="gx" + sfx, name="gx")
                    nc.scalar.activation(xg, acc[:, b, :], AF.Identity, bias=Bt)
                    sq = work.tile([8, MM], F32, tag="gs" + sfx, name="gs")
                    nc.scalar.activation(sq, acc[:, b, :], AF.Square, bias=Bt)
                    va = work.tile([8, MM], F32, tag="gv" + sfx, name="gv")
                    nc.vector.scalar_tensor_tensor(
                        va, in0=sq, scalar=1.0 / GELU_C, in1=xg,
                        op0=mybir.AluOpType.add, op1=mybir.AluOpType.mult)
                    th = work.tile([8, MM], F32, tag="gt" + sfx, name="gt")
                    nc.scalar.activation(th, va, AF.Tanh, scale=GELU_S * GELU_C)
                    nc.vector.scalar_tensor_tensor(
                        Dt[:, b, :], in0=th, scalar=1.0, in1=xg,
                        op0=mybir.AluOpType.add, op1=mybir.AluOpType.mult)

                for ci, (Xt, Wc, Bt, Dt) in enumerate(
                        ((XQ, WcQ, BQ, QD), (XK, WcK, BK, KD))):
                    acc = pp.tile([8, B, MM], F32, tag="acc", bufs=2)
                    slots = []
                    for ky in range(KS):
                        for pk in range(5):
                            slots.append((ky, 2 * pk, True))
                        slots.append((ky, 10, False))
                    for si, (ky, kx, paired) in enumerate(slots):
                        kp = 128 if paired else 64
                        rhs = bass.AP(
                            tensor=Xt.tensor,
                            offset=Xt.offset + ky * PADW + kx,
                            ap=[[Xt.ap[0][0], kp], [PADHW, B],
                                [STRIDE * PADW, M6], [STRIDE, M6]])
                        nc.tensor.matmul(acc, lhsT=Wc[0:kp, si * 8:(si + 1) * 8],
                                         rhs=rhs,
                                         start=(si == 0), stop=(si == len(slots) - 1))
                    for b in range(B):
                        gelu_chain(acc, b, Bt, Dt, f"{ci}{b}")

                for b in range(B):
                    VT = VTs[b]
                    nc.vector.memset(VT[:, :, 0:1], 1.0)
                    for g in range(2):
                        pt9 = pp.tile([128, 9, 8], BF, tag="pt", bufs=2,
                                      name="pt9")
                        for t in range(9):
                            jt = g * 9 + t
                            nc.tensor.transpose(
                                pt9[:, t, :], VV[:, b, jt * 128:(jt + 1) * 128],
                                ID8)
                        nc.vector.tensor_copy(VT[:, g * 9:(g + 1) * 9, 1:9], pt9)

                def up_build(Dt, b, scratch, scale):
                    half = MM // 2
                    for eng, lo, hi in ((nc.scalar, 0, half),
                                        (nc.vector, half, MM)):
                        dv = bass.AP(tensor=scratch.tensor,
                                     offset=scratch.offset + lo * 64,
                                     ap=[[scratch.ap[0][0], 8],
                                         [64, hi - lo], [1, 64]])
                        sv = bass.AP(tensor=Dt.tensor,
                                     offset=Dt.offset + b * MM + lo,
                                     ap=[[Dt.ap[0][0], 8], [1, hi - lo], [0, 64]])
                        if eng is nc.scalar:
                            eng.activation(dv, sv, AF.Copy, scale=scale)
                        else:
                            eng.tensor_scalar_mul(dv, sv, scale)

                for b in range(B):
                    S, R = Ss[b], Rs[b]
                    # S rows 0-7 = head-q flat, same-partition engine copy
                    nc.scalar.activation(
                        S[0:8, :].rearrange("p (h w) -> p h w", h=H, w=H),
                        interior(XQ, b)[0:8], AF.Copy)
                    nc.sync.dma_start(R[0:8, :], emb[:, :])
                    UPQ = stage.tile([8, HW], BF, tag="upq")
                    UPK = stage.tile([8, HW], BF, tag="upk")
                    up_build(QD, b, UPQ, SCALE * 0.25)
                    up_build(KD, b, UPK, 1.0)
                    nc.sync.dma_start(S[8:16, :], UPQ)
                    nc.sync.dma_start(R[8:16, :], UPK)

            with tc.tile_pool(name="psum_main", bufs=1, space="PSUM") as pm:
                steps = [(b, i0, ni, jp)
                         for b in range(B)
                         for (i0, ni) in ICHUNKS
                         for jp in range(NJT // 2)]
                po_cur = [None]
                pending = [None]

                def emit_o():
                    pb_, pi0, pni, pjp, pesb = pending[0]
                    if pjp == 0:
                        po_cur[0] = pm.tile([9, 512], F32, tag="po",
                                            bufs=2, name="po")
                    po = po_cur[0]
                    VT = VTs[pb_]
                    nc.tensor.matmul(po[:, :pni], lhsT=VT[:, 2 * pjp, :],
                                     rhs=pesb[:, 0, :pni],
                                     start=(pjp == 0), stop=False)
                    nc.tensor.matmul(po[:, :pni], lhsT=VT[:, 2 * pjp + 1, :],
                                     rhs=pesb[:, 1, :pni],
                                     start=False, stop=(pjp == NJT // 2 - 1))
                    if pjp == NJT // 2 - 1:
                        rec = work.tile([1, 512], F32, tag="rec", name="rec")
                        nc.vector.reciprocal(rec[:, :pni], po[0:1, :pni])
                        pb = pm.tile([9, 512], F32, tag="po", bufs=2, name="pb")
                        nc.tensor.matmul(pb[:, :pni], lhsT=ONE9,
                                         rhs=rec[:, :pni],
                                         start=True, stop=True)
                        pbs = work.tile([9, 512], F32, tag="pbs", name="pbs")
                        nc.vector.tensor_copy(pbs[:, :pni], pb[:, :pni])
                        res = work.tile([9, 512], F32, tag="res", name="res")
                        nc.vector.tensor_mul(res[:, :pni], po[:, :pni],
                                             pbs[:, :pni])
                        nc.sync.dma_start(out[pb_, :, pi0:pi0 + pni],
                                          res[1:9, :pni])

                for step in steps:
                    b, i0, ni, jp = step
                    S, R = Ss[b], Rs[b]
                    pe2 = pm.tile([128, 2, 512], F32, tag="pe", bufs=3,
                                  name="pe2")
                    nc.tensor.matmul(pe2[:, 0, :ni],
                                     lhsT=R[:, (2 * jp) * 128:(2 * jp + 1) * 128],
                                     rhs=S[:, i0:i0 + ni],
                                     start=True, stop=True)
                    nc.tensor.matmul(pe2[:, 1, :ni],
                                     lhsT=R[:, (2 * jp + 1) * 128:(2 * jp + 2) * 128],
                                     rhs=S[:, i0:i0 + ni],
                                     start=True, stop=True)
                    esb2 = epool.tile([128, 2, 512], BF, tag="esb", bufs=6,
                                      name="esb2")
                    nc.scalar.activation(esb2[:, :, :ni], pe2[:, :, :ni], AF.Exp)
                    if pending[0] is not None:
                        emit_o()
                    pending[0] = (b, i0, ni, jp, esb2)
                emit_o()

        for _rep in range(repeat):
            _rep_body()

    if split:
        _split_waits(nc)
    return nc


def _split_waits(nc):
    import concourse.mybir as mybir
    ctr = 0
    for fn in nc.m.functions:
        for blk in fn.blocks:
            new = []
            for inst in blk.instructions:
                si = inst.sync_info
                waits = list(si.on_wait) if si and si.on_wait else []
                if len(waits) > 1:
                    for w in waits[:-1]:
                        ctr += 1
                        nop = mybir.InstNoOp(name=f"I-wsplit-{ctr}", ins=[], outs=[])
                        nop.engine = inst.engine
                        nop.sync_info = mybir.SyncInfo(on_wait=[w], on_update=[])
                        new.append(nop)
                    inst.sync_info = mybir.SyncInfo(
                        on_wait=[waits[-1]],
                        on_update=list(si.on_update or []))
                new.append(inst)
            blk.instructions = new


def _make_in_maps(f, w_qkv, wq, bq, wk, bk, pos_h, pos_w):
    import ml_dtypes
    BF = ml_dtypes.bfloat16
    f2 = np.ascontiguousarray(f.reshape(B, C, HW)).astype(BF)
    embv = np.ascontiguousarray(
        (pos_h[:, :, None] + pos_w[:, None, :]).reshape(8, HW)).astype(BF)
    w = w_qkv[:, :, 0, 0].astype(np.float32)
    wq = wq.astype(np.float32)
    wk = wk.astype(np.float32)
    in_maps = []
    for h in range(NCORES):
        head = np.arange(h * 8, h * 8 + 8)
        rest = np.delete(np.arange(C), head)
        perm = np.concatenate([head, rest])
        wAll = np.ascontiguousarray(np.concatenate(
            [w[0:C][perm].T, w[C:2 * C].T,
             w[2 * C + h * 8: 2 * C + h * 8 + 8].T], axis=1)).astype(BF)

        def pack_taps(wp):
            w2 = np.zeros((128, 66 * 8), np.float32)
            si = 0
            for ky in range(KS):
                for pk in range(5):
                    w2[0:64, si * 8:(si + 1) * 8] = wp[:, :, ky, 2 * pk].T
                    w2[64:128, si * 8:(si + 1) * 8] = wp[:, :, ky, 2 * pk + 1].T
                    si += 1
                w2[0:64, si * 8:(si + 1) * 8] = wp[:, :, ky, 10].T
                si += 1
            return np.ascontiguousarray(w2).astype(BF)

        wqT = pack_taps(wq[head][:, perm])
        wkT = pack_taps(wk[head])
        in_maps.append({
            "f2": f2,
            "wAll": wAll,
            "wqT": wqT,
            "wkT": wkT,
            "bqk": np.ascontiguousarray(
                np.stack([bq[head], bk[head]], axis=1).astype(np.float32)),
            "emb": embv,
        })
    return in_maps


def _get_program(repeat=1):
    if repeat not in _PROGRAMS:
        _PROGRAMS[repeat] = _build_program(repeat)
    return _PROGRAMS[repeat]


def _assemble(results):
    fmap = np.empty((B, C, HW), np.float32)
    for h in range(NCORES):
        fmap[:, h * 8:(h + 1) * 8, :] = results[h]["out"]
    return fmap.reshape(B, C, H, H)


def run(trace=False, **inputs):
    from concourse.bass_utils import run_bass_kernel_spmd
    nc = _get_program()
    in_maps = _make_in_maps(**inputs)
    res = run_bass_kernel_spmd(nc, in_maps, core_ids=list(range(NCORES)),
                               trace=trace)
    return _assemble(res.results), res


def kernel(**inputs):
    out, _ = run(trace=False, **inputs)
    return out


# revision 31
# speedup vs baseline: 1.2903x; 1.0208x over previous
"""v4 + batch-1 S/R finishing deferred into Main0 (DVE/SP only)."""

import numpy as np

HEADS = 8
DIM_HEAD = 8
B = 2
C = 64
H = 48
HW = H * H
KS = 11
STRIDE = 8
PAD = 2
M6 = 6
MM = M6 * M6
PADW = H + 2 * PAD
PADHW = PADW * PADW
SCALE = DIM_HEAD ** (-0.5)
NCORES = 8
TAPS = KS * KS

ICHUNKS = [(0, 512), (512, 512), (1024, 512), (1536, 512), (2048, 256)]
QCHUNKS = [(0, 480), (480, 480), (960, 480), (1440, 480), (1920, 384)]
NJT = HW // 128

_PROGRAMS = {}
_CONV_ACT = "Gelu"
_DEBUG = False


def _build_program(repeat=1, split=True):
    from contextlib import ExitStack
    import concourse.bass as bass
    import concourse.mybir as mybir
    import concourse.tile as tile
    from concourse.masks import make_identity

    F32 = mybir.dt.float32
    BF = mybir.dt.bfloat16
    AF = mybir.ActivationFunctionType

    nc = bass.Bass(trn_type="TRN2")

    f2 = nc.declare_dram_parameter("f2", [B, C, HW], BF, isOutput=False)
    wAll = nc.declare_dram_parameter("wAll", [C, 136], BF, isOutput=False)
    wqT = nc.declare_dram_parameter("wqT", [128, 66 * 8], BF, isOutput=False)
    wkT = nc.declare_dram_parameter("wkT", [128, 66 * 8], BF, isOutput=False)
    bqk = nc.declare_dram_parameter("bqk", [8, 2], F32, isOutput=False)
    emb = nc.declare_dram_parameter("emb", [8, HW], BF, isOutput=False)
    out = nc.declare_dram_parameter("out", [B, 8, HW], F32, isOutput=True)

    def interior(Xt, b):
        return bass.AP(
            tensor=Xt.tensor,
            offset=Xt.offset + b * PADHW + PAD * PADW + PAD,
            ap=[Xt.ap[0], [PADW, H], [1, H]],
        )

    def up_ap(Dt, b, nrows=8):
        return bass.AP(
            tensor=Dt.tensor,
            offset=Dt.offset + b * MM,
            ap=[[Dt.ap[0][0], nrows], [1, MM], [0, 64]],
        )

    with tile.TileContext(nc) as tc, ExitStack() as ctx:
        const = ctx.enter_context(tc.tile_pool(name="const", bufs=1))
        work = ctx.enter_context(tc.tile_pool(name="work", bufs=3))
        epool = ctx.enter_context(tc.tile_pool(name="epool", bufs=3))

        ID8 = const.tile([8, 8], BF)
        make_identity(nc, ID8)
        ONE9 = const.tile([1, 9], F32)
        nc.vector.memset(ONE9, 1.0)

        def _rep_body():
            WA = const.tile([C, 136], BF)
            nc.sync.dma_start(WA, wAll[:, :])
            W1 = WA[:, 0:128]
            WV = WA[:, 128:136]
            BQK = const.tile([8, 2], F32)
            nc.sync.dma_start(BQK, bqk[:, :])
            BQ = BQK[:, 0:1]
            BK = BQK[:, 1:2]
            WcQ = const.tile([128, 66 * 8], BF)
            nc.sync.dma_start(WcQ, wqT[:, :])
            WcK = const.tile([128, 66 * 8], BF)
            nc.sync.dma_start(WcK, wkT[:, :])
            QD = const.tile([8, B, MM], F32)
            KD = const.tile([8, B, MM], F32)
            Ss = [const.tile([16, HW], BF, name=f"S{b}") for b in range(B)]
            Rs = [const.tile([16, HW], BF, name=f"R{b}") for b in range(B)]
            VTs = [const.tile([128, NJT, 9], BF, name=f"VT{b}") for b in range(B)]

            with tc.tile_pool(name="stage", bufs=1) as stage, \
                 tc.tile_pool(name="psum_pro", bufs=1, space="PSUM") as pp:
                F = stage.tile([C, B, HW], BF)
                for b in range(B):
                    nc.sync.dma_start(F[:, b, :], f2[b, :, :])
                XQ = stage.tile([128, B, PADHW], BF)
                XK = stage.tile([128, B, PADHW], BF)
                VV = stage.tile([8, B, HW], BF)

                for Xt in (XQ, XK):
                    for b in range(B):
                        base = b * PADHW
                        for shift, poff in ((0, 0), (1, 64)):
                            o = Xt.offset + poff * Xt.ap[0][0] + base
                            pap = [[Xt.ap[0][0], 64]]
                            nc.gpsimd.memset(
                                bass.AP(tensor=Xt.tensor, offset=o,
                                        ap=pap + [[1, 2 * PADW + PAD - shift]]), 0.0)
                            nc.gpsimd.memset(
                                bass.AP(tensor=Xt.tensor,
                                        offset=o + (H + PAD - 1) * PADW + PAD + H - shift,
                                        ap=pap + [[1, 2 * PADW + PAD + shift]]), 0.0)
                            nc.gpsimd.memset(
                                bass.AP(tensor=Xt.tensor,
                                        offset=o + PAD * PADW + PAD + H - shift,
                                        ap=pap + [[PADW, H - 1], [1, 2 * PAD]]), 0.0)

                for b in range(B):
                    for (j0, nj) in QCHUNKS:
                        nrows = nj // H
                        y0 = j0 // H
                        pq = pp.tile([128, 480], F32, tag="pq", bufs=2)
                        nc.tensor.matmul(pq[:, :nj], lhsT=W1, rhs=F[:, b, j0:j0 + nj],
                                         start=True, stop=True)
                        pv = pp.tile([8, 480], F32, tag="pv", bufs=2)
                        nc.tensor.matmul(pv[:, :nj], lhsT=WV, rhs=F[:, b, j0:j0 + nj],
                                         start=True, stop=True)
                        for Xt, r0, eng in ((XQ, 0, nc.scalar),
                                            (XK, 64, nc.vector)):
                            src = pq[r0:r0 + 64, :nj].rearrange(
                                "p (r w) -> p r w", r=nrows, w=H)
                            for shift, poff in ((0, 0), (1, 64)):
                                dst = bass.AP(
                                    tensor=Xt.tensor,
                                    offset=(Xt.offset + poff * Xt.ap[0][0]
                                            + b * PADHW
                                            + (PAD + y0) * PADW + PAD - shift),
                                    ap=[[Xt.ap[0][0], 64], [PADW, nrows], [1, H]])
                                if eng is nc.scalar:
                                    eng.activation(dst, src, AF.Copy)
                                else:
                                    eng.tensor_copy(dst, src)
                        nc.vector.tensor_copy(VV[:, b, j0:j0 + nj], pv[:, :nj])

                GELU_C = 0.044715
                GELU_S = 0.7978845608028654

                def gelu_chain(acc, b, Bt, Dt, sfx):
                    xg = work.tile([8, MM], F32, tag="gx" + sfx, name="gx")
                    nc.scalar.activation(xg, acc[:, b, :], AF.Identity, bias=Bt)
                    sq = work.tile([8, MM], F32, tag="gs" + sfx, name="gs")
                    nc.scalar.activation(sq, acc[:, b, :], AF.Square, bias=Bt)
                    va = work.tile([8, MM], F32, tag="gv" + sfx, name="gv")
                    nc.vector.scalar_tensor_tensor(
                        va, in0=sq, scalar=1.0 / GELU_C, in1=xg,
                        op0=mybir.AluOpType.add, op1=mybir.AluOpType.mult)
                    th = work.tile([8, MM], F32, tag="gt" + sfx, name="gt")
                    nc.scalar.activation(th, va, AF.Tanh, scale=GELU_S * GELU_C)
                    nc.vector.scalar_tensor_tensor(
                        Dt[:, b, :], in0=th, scalar=1.0, in1=xg,
                        op0=mybir.AluOpType.add, op1=mybir.AluOpType.mult)

                for ci, (Xt, Wc, Bt, Dt) in enumerate(
                        ((XQ, WcQ, BQ, QD), (XK, WcK, BK, KD))):
                    acc = pp.tile([8, B, MM], F32, tag="acc", bufs=2)
                    slots = []
                    for ky in range(KS):
                        for pk in range(5):
                            slots.append((ky, 2 * pk, True))
                        slots.append((ky, 10, False))
                    for si, (ky, kx, paired) in enumerate(slots):
                        kp = 128 if paired else 64
                        rhs = bass.AP(
                            tensor=Xt.tensor,
                            offset=Xt.offset + ky * PADW + kx,
                            ap=[[Xt.ap[0][0], kp], [PADHW, B],
                                [STRIDE * PADW, M6], [STRIDE, M6]])
                        nc.tensor.matmul(acc, lhsT=Wc[0:kp, si * 8:(si + 1) * 8],
                                         rhs=rhs,
                                         start=(si == 0), stop=(si == len(slots) - 1))
                    for b in range(B):
                        gelu_chain(acc, b, Bt, Dt, f"{ci}{b}")

                for b in range(B):
                    VT = VTs[b]
                    nc.vector.memset(VT[:, :, 0:1], 1.0)
                    for g in range(2):
                        pt9 = pp.tile([128, 9, 8], BF, tag="pt", bufs=2,
                                      name="pt9")
                        for t in range(9):
                            jt = g * 9 + t
                            nc.tensor.transpose(
                                pt9[:, t, :], VV[:, b, jt * 128:(jt + 1) * 128],
                                ID8)
                        nc.vector.tensor_copy(VT[:, g * 9:(g + 1) * 9, 1:9], pt9)

                def up_build(Dt, b, scratch, scale, dve_only=False):
                    half = MM // 2
                    engs = ((nc.vector, 0, half), (nc.vector, half, MM)) \
                        if dve_only else ((nc.scalar, 0, half),
                                          (nc.vector, half, MM))
                    for eng, lo, hi in engs:
                        dv = bass.AP(tensor=scratch.tensor,
                                     offset=scratch.offset + lo * 64,
                                     ap=[[scratch.ap[0][0], 8],
                                         [64, hi - lo], [1, 64]])
                        sv = bass.AP(tensor=Dt.tensor,
                                     offset=Dt.offset + b * MM + lo,
                                     ap=[[Dt.ap[0][0], 8], [1, hi - lo], [0, 64]])
                        if eng is nc.scalar:
                            eng.activation(dv, sv, AF.Copy, scale=scale)
                        else:
                            eng.tensor_scalar_mul(dv, sv, scale)

                def build_sr(b, dve_only=False):
                    S, R = Ss[b], Rs[b]
                    if dve_only:
                        nc.vector.tensor_copy(
                            S[0:8, :].rearrange("p (h w) -> p h w", h=H, w=H),
                            interior(XQ, b)[0:8])
                    else:
                        nc.scalar.activation(
                            S[0:8, :].rearrange("p (h w) -> p h w", h=H, w=H),
                            interior(XQ, b)[0:8], AF.Copy)
                    nc.sync.dma_start(R[0:8, :], emb[:, :])
                    UPQ = stage.tile([8, HW], BF, tag="upq")
                    UPK = stage.tile([8, HW], BF, tag="upk")
                    up_build(QD, b, UPQ, SCALE * 0.25, dve_only)
                    up_build(KD, b, UPK, 1.0, dve_only)
                    nc.sync.dma_start(S[8:16, :], UPQ)
                    nc.sync.dma_start(R[8:16, :], UPK)

                build_sr(0)

                deferred = [lambda: build_sr(1, dve_only=True)]
                _main(deferred)

        def _main_pools():
            pass

                steps = [(b, i0, ni, jp)
                         for b in range(B)
                         for (i0, ni) in ICHUNKS
                         for jp in range(NJT // 2)]
                po_cur = [None]
                pending = [None]

                def emit_o():
                    pb_, pi0, pni, pjp, pesb = pending[0]
                    if pjp == 0:
                        po_cur[0] = pm.tile([9, 512], F32, tag="po",
                                            bufs=2, name="po")
                    po = po_cur[0]
                    VT = VTs[pb_]
                    nc.tensor.matmul(po[:, :pni], lhsT=VT[:, 2 * pjp, :],
                                     rhs=pesb[:, 0, :pni],
                                     start=(pjp == 0), stop=False)
                    nc.tensor.matmul(po[:, :pni], lhsT=VT[:, 2 * pjp + 1, :],
                                     rhs=pesb[:, 1, :pni],
                                     start=False, stop=(pjp == NJT // 2 - 1))
                    if pjp == NJT // 2 - 1:
                        rec = work.tile([1, 512], F32, tag="rec", name="rec")
                        nc.vector.reciprocal(rec[:, :pni], po[0:1, :pni])
                        pbs = work.tile([9, 512], F32, tag="pbs", name="pbs")
                        if (pb_, pi0) == (B - 1, ICHUNKS[-1][0]):
                            pbp = pm.tile([9, 512], F32, tag="po", bufs=2,
                                          name="pbp")
                            nc.tensor.matmul(pbp[:, :pni], lhsT=ONE9,
                                             rhs=rec[:, :pni],
                                             start=True, stop=True)
                            nc.vector.tensor_copy(pbs[:, :pni], pbp[:, :pni])
                        else:
                            nc.sync.dma_start(
                                pbs[:, :pni],
                                bass.AP(tensor=rec.tensor, offset=rec.offset,
                                        ap=[[1, 1], [0, 9], [1, pni]]))
                        res = work.tile([9, 512], F32, tag="res", name="res")
                        nc.vector.tensor_mul(res[:, :pni], po[:, :pni],
                                             pbs[:, :pni])
                        nc.sync.dma_start(out[pb_, :, pi0:pi0 + pni],
                                          res[1:9, :pni])

                for step in steps:
                    b, i0, ni, jp = step
                    S, R = Ss[b], Rs[b]
                    pe2 = pm.tile([128, 2, 512], F32, tag="pe", bufs=3,
                                  name="pe2")
                    nc.tensor.matmul(pe2[:, 0, :ni],
                                     lhsT=R[:, (2 * jp) * 128:(2 * jp + 1) * 128],
                                     rhs=S[:, i0:i0 + ni],
                                     start=True, stop=True)
                    nc.tensor.matmul(pe2[:, 1, :ni],
                                     lhsT=R[:, (2 * jp + 1) * 128:(2 * jp + 2) * 128],
                                     rhs=S[:, i0:i0 + ni],
                                     start=True, stop=True)
                    esb2 = epool.tile([128, 2, 512], BF, tag="esb", bufs=8,
                                      name="esb2")
                    nc.scalar.activation(esb2[:, :, :ni], pe2[:, :, :ni], AF.Exp)
                    if pending[0] is not None:
                        emit_o()
                    pending[0] = (b, i0, ni, jp, esb2)
                emit_o()

        for _rep in range(repeat):
            _rep_body()

    if split:
        _split_waits(nc)
    return nc


def _split_waits(nc):
    import concourse.mybir as mybir
    ctr = 0
    for fn in nc.m.functions:
        for blk in fn.blocks:
            new = []
            for inst in blk.instructions:
                si = inst.sync_info
                waits = list(si.on_wait) if si and si.on_wait else []
                if len(waits) > 1:
                    for w in waits[:-1]:
                        ctr += 1
                        nop = mybir.InstNoOp(name=f"I-wsplit-{ctr}", ins=[], outs=[])
                        nop.engine = inst.engine
                        nop.sync_info = mybir.SyncInfo(on_wait=[w], on_update=[])
                        new.append(nop)
                    inst.sync_info = mybir.SyncInfo(
                        on_wait=[waits[-1]],
                        on_update=list(si.on_update or []))
                new.append(inst)
            blk.instructions = new


def _make_in_maps(f, w_qkv, wq, bq, wk, bk, pos_h, pos_w):
    import ml_dtypes
    BF = ml_dtypes.bfloat16
    f2 = np.ascontiguousarray(f.reshape(B, C, HW)).astype(BF)
    embv = np.ascontiguousarray(
        (pos_h[:, :, None] + pos_w[:, None, :]).reshape(8, HW)).astype(BF)
    w = w_qkv[:, :, 0, 0].astype(np.float32)
    wq = wq.astype(np.float32)
    wk = wk.astype(np.float32)
    in_maps = []
    for h in range(NCORES):
        head = np.arange(h * 8, h * 8 + 8)
        rest = np.delete(np.arange(C), head)
        perm = np.concatenate([head, rest])
        wAll = np.ascontiguousarray(np.concatenate(
            [w[0:C][perm].T, w[C:2 * C].T,
             w[2 * C + h * 8: 2 * C + h * 8 + 8].T], axis=1)).astype(BF)

        def pack_taps(wp):
            w2 = np.zeros((128, 66 * 8), np.float32)
            si = 0
            for ky in range(KS):
                for pk in range(5):
                    w2[0:64, si * 8:(si + 1) * 8] = wp[:, :, ky, 2 * pk].T
                    w2[64:128, si * 8:(si + 1) * 8] = wp[:, :, ky, 2 * pk + 1].T
                    si += 1
                w2[0:64, si * 8:(si + 1) * 8] = wp[:, :, ky, 10].T
                si += 1
            return np.ascontiguousarray(w2).astype(BF)

        wqT = pack_taps(wq[head][:, perm])
        wkT = pack_taps(wk[head])
        in_maps.append({
            "f2": f2,
            "wAll": wAll,
            "wqT": wqT,
            "wkT": wkT,
            "bqk": np.ascontiguousarray(
                np.stack([bq[head], bk[head]], axis=1).astype(np.float32)),
            "emb": embv,
        })
    return in_maps


def _get_program(repeat=1):
    if repeat not in _PROGRAMS:
        _PROGRAMS[repeat] = _build_program(repeat)
    return _PROGRAMS[repeat]


def _assemble(results):
    fmap = np.empty((B, C, HW), np.float32)
    for h in range(NCORES):
        fmap[:, h * 8:(h + 1) * 8, :] = results[h]["out"]
    return fmap.reshape(B, C, H, H)


def run(trace=False, **inputs):
    from concourse.bass_utils import run_bass_kernel_spmd
    nc = _get_program()
    in_maps = _make_in_maps(**inputs)
    res = run_bass_kernel_spmd(nc, in_maps, core_ids=list(range(NCORES)),
                               trace=trace)
    return _assemble(res.results), res


def kernel(**inputs):
    out, _ = run(trace=False, **inputs)
    return out
